# revision 42
# baseline (speedup 1.0000x reference)
"""Paged prefill attention (sparse_attention) on 8 Trainium2 NeuronCores.

Problem (hardcoded, mirrors the reference):
  q:        [2048, 32, 128] f32   (2 seqs x 1024 query tokens, 32 heads)
  k_cache:  [64, 64, 8, 128] f32  (64 physical blocks x 64 tokens x 8 kv heads)
  v_cache:  [64, 64, 8, 128] f32
  cu_seqlens_q: [0, 1024, 2048]
  cu_seqlens_k: [0, 2048, 4096]
  block_tables: [2, 32] int32 permutation of the 64 physical blocks
  out:      [2048, 32, 128] f32

Sharding: tensor-parallel by kv head. Core h gets kv head h plus its 4
query heads (GQA group 4), both full sequences (SPMD, one program).

Host-side prep (inside kernel(), per core): the block-table gather, the
per-head shard, the fp32->fp16 cast and the on-chip layouts are all done
in numpy so the device program is pure compute + bulk DMA:
  qT [128=d, 8*1024]  fp16  (d on partitions, col = (s*4+h)*1024 + tok)
  kT [128=d, 2*2048]  fp16  (col = s*2048 + tok, block table applied)
  vP [128=tok, 2*16*129] fp16 (chunk-major, 129th column = 1.0 ones)
  out [128=tok%128, 2*4*8*128] f32 (col = ((s*4+h)*8 + qt)*128 + d)

Device per (seq, head) unit (fp16 matmuls, S^T flash layout):
  - QK: S^T[k,q] = kT_chunk.T @ qT into PSUM [128, width<=1024],
    causally clipped per 128-token kv chunk.
  - exp(scale*s) from PSUM into fp16 es; split between the ACT engine
    (exact Exp activation) and the DVE engine (Schraudolph bit-trick:
    one fused tensor_scalar f32->int16, bitcast to fp16) so no single
    engine exceeds the PE roofline.
  - diagonal 128x128 blocks zeroed (upper triangle) on GPSIMD.
  - PV: po[q, 0:129] += es_chunk.T @ vP_chunk; col 128 (ones) is the
    softmax denominator.
  - normalize: DVE reciprocal + per-partition scalar multiply, bulk DMA.
PV of unit u-1 is interleaved between QK chunks of unit u so the
in-order PE never stalls on PSUM recycling or exp latency.
"""

import numpy as np

NUM_SEQS = 2
LQ = 1024
HIST = 1024
LK = LQ + HIST
NUM_HEADS = 32
NUM_KV_HEADS = 8
GROUP = NUM_HEADS // NUM_KV_HEADS  # 4 q heads per kv head / core
HEAD_DIM = 128
BLOCK_SIZE = 64
NBLK = LK // BLOCK_SIZE         # 32 logical blocks per sequence
TOTAL_BLOCKS = NUM_SEQS * NBLK  # 64 physical blocks
NCH = LK // 128                 # 16 128-token kv chunks per sequence
NQT = LQ // 128                 # 8 128-token q tiles per sequence
NU = NUM_SEQS * GROUP           # 8 (seq, head) units per core
SCALE = 1.0 / float(np.sqrt(HEAD_DIM))

# exp engine split: these kv chunks go to DVE (Schraudolph bit-trick),
# the rest to ACT (exact). ~24% of exp columns on DVE.
DVE_KT = frozenset({2, 6, 10, 12, 14})
# unit 0 (pipeline fill) alternates engines for exp latency, not accuracy
U0_DVE_KT = frozenset({1, 3, 5, 7, 9, 11, 13, 15})
# Schraudolph fp16 exp: es = bitcast_f16(int16(s * EXP_A + EXP_B))
EXP_A = float(SCALE * np.log2(np.e) * 1024.0)
EXP_B = float(15 * 1024 - 45)

_CACHE = {}


def _build_program():
    from contextlib import ExitStack

    import concourse.mybir as mybir
    import concourse.tile as tile
    from concourse import bacc

    f32 = mybir.dt.float32
    f16 = mybir.dt.float16
    i16 = mybir.dt.int16

    nc = bacc.Bacc()
    qT_d = nc.dram_tensor("qT", [128, NU * LQ], f16, kind="ExternalInput")
    kT_d = nc.dram_tensor("kT", [128, NUM_SEQS * LK], f16, kind="ExternalInput")
    vP_d = nc.dram_tensor("vP", [128, NUM_SEQS * NCH * 129], f16,
                          kind="ExternalInput")
    o_d = nc.dram_tensor("out", [128, NU * NQT * 128], f32,
                         kind="ExternalOutput")

    with tile.TileContext(nc) as tc, ExitStack() as ctx:
        persist = ctx.enter_context(tc.tile_pool(name="persist", bufs=1))
        es_pool = ctx.enter_context(tc.tile_pool(name="es", bufs=3))
        ob_pool = ctx.enter_context(tc.tile_pool(name="ob", bufs=3))
        small = ctx.enter_context(tc.tile_pool(name="small", bufs=8))
        sc_ps = ctx.enter_context(tc.tile_pool(name="sc_ps", bufs=3, space="PSUM"))
        oc_ps = ctx.enter_context(tc.tile_pool(name="oc_ps", bufs=2, space="PSUM"))

        qTs = persist.tile([128, NU * LQ], f16, tag="qTs")
        kTs = persist.tile([128, NUM_SEQS * LK], f16, tag="kTs")
        vPs = persist.tile([128, NUM_SEQS * NCH * 129], f16, tag="vPs")

        # warmup: load the Exp activation table while the input DMAs run
        warm = persist.tile([128, 1], f32, tag="warm")
        warm16 = persist.tile([128, 1], f16, tag="warm16")
        nc.vector.memset(warm[:, :], 0.0)
        nc.scalar.activation(warm16[:, :], warm[:, :],
                             mybir.ActivationFunctionType.Exp, scale=SCALE)
        # dummy matmul burst: keeps PE continuously busy through the DMA
        # fill so the p-state ramp completes before the first real QK
        warmS = persist.tile([128, 1], f16, tag="warmS")
        scratch = persist.tile([128, 64], f16, tag="scratch")
        nc.vector.memset(warmS[:, :], 0.0)
        nc.vector.memset(scratch[:, :], 0.0)
        warmP = sc_ps.tile([128, 1024], f32, tag="sc")
        for _ in range(34):
            nc.tensor.matmul(warmP[0:1, 0:64], warmS[:, :], scratch[:, :],
                             start=True, stop=True)

        # input DMAs, ordered so unit 0's dependencies land first
        nc.sync.dma_start(out=kTs[:, 0:256], in_=kT_d[:, 0:256])
        nc.sync.dma_start(out=qTs[:, 0:512], in_=qT_d[:, 0:512])
        nc.sync.dma_start(out=qTs[:, 512:LQ], in_=qT_d[:, 512:LQ])
        nc.sync.dma_start(out=kTs[:, 256:LK // 2], in_=kT_d[:, 256:LK // 2])
        nc.sync.dma_start(out=kTs[:, LK // 2:LK], in_=kT_d[:, LK // 2:LK])
        nc.sync.dma_start(out=vPs[:, 0:NCH * 129], in_=vP_d[:, 0:NCH * 129])
        for h in range(1, GROUP):
            nc.sync.dma_start(out=qTs[:, h * LQ:(h + 1) * LQ],
                              in_=qT_d[:, h * LQ:(h + 1) * LQ])
        nc.sync.dma_start(out=kTs[:, LK:2 * LK], in_=kT_d[:, LK:2 * LK])
        nc.sync.dma_start(out=vPs[:, NCH * 129:2 * NCH * 129],
                          in_=vP_d[:, NCH * 129:2 * NCH * 129])
        for h in range(GROUP):
            u = GROUP + h
            nc.sync.dma_start(out=qTs[:, u * LQ:(u + 1) * LQ],
                              in_=qT_d[:, u * LQ:(u + 1) * LQ])

        def dve_kt(u):
            # unit 0 paces the pipeline fill on exp throughput: alternate
            # ACT/DVE there; steady-state units use the accuracy-tuned set
            return U0_DVE_KT if u == 0 else DVE_KT

        def emit_qk_chunk(u, kt, es):
            s = u // GROUP
            q_lo = max(0, (kt - NCH // 2) * 128)
            width = LQ - q_lo
            ps = sc_ps.tile([128, 1024], f32, tag="sc")
            off = 0
            while off < width:
                n = min(512 - off % 512, width - off)
                nc.tensor.matmul(
                    ps[:, off:off + n],
                    kTs[:, s * LK + kt * 128:s * LK + (kt + 1) * 128],
                    qTs[:, u * LQ + q_lo + off:u * LQ + q_lo + off + n],
                    start=True, stop=True)
                off += n
            dst_lo = kt * LQ + q_lo
            if kt in dve_kt(u):
                nc.vector.tensor_scalar(
                    es[:, dst_lo:(kt + 1) * LQ].bitcast(i16),
                    ps[:, 0:width], EXP_A, EXP_B,
                    mybir.AluOpType.mult, mybir.AluOpType.add)
            else:
                nc.scalar.activation(
                    es[:, dst_lo:(kt + 1) * LQ], ps[:, 0:width],
                    mybir.ActivationFunctionType.Exp, scale=SCALE)
            if kt >= NCH // 2:
                # zero the upper triangle of the diagonal 128x128 block
                nc.gpsimd.affine_select(
                    out=es[:, dst_lo:dst_lo + 128],
                    in_=es[:, dst_lo:dst_lo + 128],
                    compare_op=mybir.AluOpType.is_ge, fill=0.0,
                    base=0, pattern=[[1, 128]], channel_multiplier=-1)

        def emit_pv_qt(u, qt, es, ob8):
            s = u // GROUP
            nch_q = NCH // 2 + 1 + qt  # kv chunks 0 .. 8+qt
            po = oc_ps.tile([128, 129], f32, tag="oc")
            for c in range(nch_q):
                nc.tensor.matmul(
                    po[:, :],
                    es[:, c * LQ + qt * 128:c * LQ + (qt + 1) * 128],
                    vPs[:, (s * NCH + c) * 129:(s * NCH + c + 1) * 129],
                    start=(c == 0), stop=(c == nch_q - 1))
            rc = small.tile([128, 1], f32, tag="rc")
            nc.vector.reciprocal(rc[:, :], po[:, 128:129])
            nc.vector.tensor_scalar_mul(
                ob8[:, qt * 128:(qt + 1) * 128], po[:, 0:128], rc[:, :])

        def emit_out_dma(u, ob8, lo=0, hi=NQT):
            nc.sync.dma_start(
                out=o_d[:, (u * NQT + lo) * 128:(u * NQT + hi) * 128],
                in_=ob8[:, lo * 128:hi * 128])

        # software pipeline: QK/exp of unit u interleaved with PV of u-1;
        # the last unit's PV is folded into its own QK loop (2-chunk lag)
        # so the tail is short. Unit 1 starts PV(u0) later (kt=6) to give
        # unit 0's exp more slack during pipeline fill.
        prev = None  # (u, es, ob8)
        for u in range(NU):
            last = u == NU - 1
            lag = 6 if u == 1 else 3
            es = es_pool.tile([128, NCH * LQ], f16, tag="es")
            ob8 = ob_pool.tile([128, NQT * 128], f32, tag="ob8")
            # first `lag` chunks fill the PSUM pipeline. For unit 0 the
            # first two chunks interleave half-width matmuls so the first
            # work depends only on the earliest DMA pieces.
            if u == 0:
                ps0 = sc_ps.tile([128, 1024], f32, tag="sc")
                ps1 = sc_ps.tile([128, 1024], f32, tag="sc")
                for half in range(2):
                    for kt, ps in ((0, ps0), (1, ps1)):
                        nc.tensor.matmul(
                            ps[:, half * 512:(half + 1) * 512],
                            kTs[:, kt * 128:(kt + 1) * 128],
                            qTs[:, half * 512:(half + 1) * 512],
                            start=True, stop=True)
                for kt, ps in ((0, ps0), (1, ps1)):
                    if kt in dve_kt(0):
                        nc.vector.tensor_scalar(
                            es[:, kt * LQ:(kt + 1) * LQ].bitcast(i16),
                            ps[:, :], EXP_A, EXP_B,
                            mybir.AluOpType.mult, mybir.AluOpType.add)
                    else:
                        nc.scalar.activation(
                            es[:, kt * LQ:(kt + 1) * LQ], ps[:, :],
                            mybir.ActivationFunctionType.Exp, scale=SCALE)
                for kt in range(2, lag):
                    emit_qk_chunk(u, kt, es)
            else:
                for kt in range(lag):
                    emit_qk_chunk(u, kt, es)
            for kt in range(lag, NCH):
                if prev is not None and kt - lag < NQT:
                    emit_pv_qt(prev[0], kt - lag, prev[1], prev[2])
                    if kt - lag == NQT // 2 - 1:
                        emit_out_dma(prev[0], prev[2], 0, NQT // 2)
                emit_qk_chunk(u, kt, es)
                if last and kt >= 10:
                    emit_pv_qt(u, kt - 10, es, ob8)
                    if kt - 10 == NQT // 2 - 1:
                        emit_out_dma(u, ob8, 0, NQT // 2)
            if prev is not None:
                for qt in range(NCH - lag, NQT):
                    emit_pv_qt(prev[0], qt, prev[1], prev[2])
                emit_out_dma(prev[0], prev[2], NQT // 2, NQT)
            prev = (u, es, ob8)
        for qt in range(NCH - 10, NQT):
            emit_pv_qt(prev[0], qt, prev[1], prev[2])
            if qt == 6:
                emit_out_dma(prev[0], prev[2], 4, 7)
        emit_out_dma(prev[0], prev[2], 7, NQT)

    nc.compile()
    return nc


def _get_program():
    if "p" not in _CACHE:
        _CACHE["p"] = _build_program()
    return _CACHE["p"]


def _host_prep(q, k_cache, v_cache, bt):
    """Build per-core device-layout inputs (gather + shard + cast)."""
    f16 = np.float16
    # qT: [128=d, s, h, t] per core
    qf = np.ascontiguousarray(
        q.reshape(NUM_SEQS, LQ, NUM_HEADS, HEAD_DIM).transpose(3, 0, 2, 1)
    ).astype(f16)  # [128, s, H, t]
    # gather blocks in logical order
    kg = k_cache[bt.reshape(-1)]  # [64, 64, KVH, 128]
    vg = v_cache[bt.reshape(-1)]
    # kT: [KVH][128=d, s*2048 + tok]
    kT_all = np.ascontiguousarray(
        kg.reshape(NUM_SEQS, LK, NUM_KV_HEADS, HEAD_DIM).transpose(2, 3, 0, 1)
    ).astype(f16)  # [KVH, 128, s, 2048]
    # vP: [KVH][128=tok%128, (s*16+c)*129 + d], col 128 = ones
    vr = vg.reshape(NUM_SEQS, NCH, 128, NUM_KV_HEADS, HEAD_DIM)
    vP_all = np.ones((NUM_KV_HEADS, 128, NUM_SEQS, NCH, 129), dtype=f16)
    vP_all[..., 0:128] = vr.transpose(3, 2, 0, 1, 4).astype(f16)

    in_maps = []
    for c in range(NUM_KV_HEADS):
        in_maps.append({
            "qT": np.ascontiguousarray(
                qf[:, :, c * GROUP:(c + 1) * GROUP, :]).reshape(128, NU * LQ),
            "kT": kT_all[c].reshape(128, NUM_SEQS * LK),
            "vP": np.ascontiguousarray(vP_all[c]).reshape(
                128, NUM_SEQS * NCH * 129),
        })
    return in_maps


def kernel(q, k_cache, v_cache, cu_seqlens_q, cu_seqlens_k, block_tables,
           _want_trace=False):
    from concourse import bass_utils

    q = np.asarray(q, dtype=np.float32)
    k_cache = np.asarray(k_cache, dtype=np.float32)
    v_cache = np.asarray(v_cache, dtype=np.float32)
    bt = np.asarray(block_tables, dtype=np.int32)

    assert q.shape == (NUM_SEQS * LQ, NUM_HEADS, HEAD_DIM)
    assert k_cache.shape == (TOTAL_BLOCKS, BLOCK_SIZE, NUM_KV_HEADS, HEAD_DIM)
    assert v_cache.shape == (TOTAL_BLOCKS, BLOCK_SIZE, NUM_KV_HEADS, HEAD_DIM)
    assert bt.shape == (NUM_SEQS, NBLK)
    assert bt.min() >= 0

    nc = _get_program()
    in_maps = _host_prep(q, k_cache, v_cache, bt)

    res = bass_utils.run_bass_kernel_spmd(
        nc, in_maps, core_ids=list(range(NUM_KV_HEADS)),
        trace=_want_trace,
        **({"trace_cores": list(range(NUM_KV_HEADS)), "stitch_traces": True}
           if _want_trace else {}),
    )

    out = np.empty((NUM_SEQS * LQ, NUM_HEADS, HEAD_DIM), dtype=np.float32)
    for c in range(NUM_KV_HEADS):
        # device layout [128=t, ((s*4+h)*8 + qt)*128 + d]
        r = res.results[c]["out"].reshape(128, NUM_SEQS, GROUP, NQT, HEAD_DIM)
        out[:, c * GROUP:(c + 1) * GROUP, :] = (
            r.transpose(1, 3, 0, 2, 4).reshape(NUM_SEQS * LQ, GROUP, HEAD_DIM))

    if _want_trace:
        return out, res
    return out


# revision 44
# speedup vs baseline: 1.0082x; 1.0082x over previous
"""Paged prefill attention (sparse_attention) on 8 Trainium2 NeuronCores.

Problem (hardcoded, mirrors the reference):
  q:        [2048, 32, 128] f32   (2 seqs x 1024 query tokens, 32 heads)
  k_cache:  [64, 64, 8, 128] f32  (64 physical blocks x 64 tokens x 8 kv heads)
  v_cache:  [64, 64, 8, 128] f32
  cu_seqlens_q: [0, 1024, 2048]
  cu_seqlens_k: [0, 2048, 4096]
  block_tables: [2, 32] int32 permutation of the 64 physical blocks
  out:      [2048, 32, 128] f32

Sharding: tensor-parallel by kv head. Core h gets kv head h plus its 4
query heads (GQA group 4), both full sequences (SPMD, one program).

Host-side prep (inside kernel(), per core): the block-table gather, the
per-head shard, the fp32->fp16 cast and the on-chip layouts are all done
in numpy so the device program is pure compute + bulk DMA:
  qT [128=d, 8*1024]  fp16  (d on partitions, col = (s*4+h)*1024 + tok)
  kT [128=d, 2*2048]  fp16  (col = s*2048 + tok, block table applied)
  vP [128=tok, 2*16*129] fp16 (chunk-major, 129th column = 1.0 ones)
  out [128=tok%128, 2*4*8*128] f32 (col = ((s*4+h)*8 + qt)*128 + d)

Device per (seq, head) unit (fp16 matmuls, S^T flash layout):
  - QK: S^T[k,q] = kT_chunk.T @ qT into PSUM [128, width<=1024],
    causally clipped per 128-token kv chunk.
  - exp(scale*s) from PSUM into fp16 es; split between the ACT engine
    (exact Exp activation) and the DVE engine (Schraudolph bit-trick:
    one fused tensor_scalar f32->int16, bitcast to fp16) so no single
    engine exceeds the PE roofline.
  - diagonal 128x128 blocks zeroed (upper triangle) on GPSIMD.
  - PV: po[q, 0:129] += es_chunk.T @ vP_chunk; col 128 (ones) is the
    softmax denominator.
  - normalize: DVE reciprocal + per-partition scalar multiply, bulk DMA.
PV of unit u-1 is interleaved between QK chunks of unit u so the
in-order PE never stalls on PSUM recycling or exp latency.
"""

import numpy as np

NUM_SEQS = 2
LQ = 1024
HIST = 1024
LK = LQ + HIST
NUM_HEADS = 32
NUM_KV_HEADS = 8
GROUP = NUM_HEADS // NUM_KV_HEADS  # 4 q heads per kv head / core
HEAD_DIM = 128
BLOCK_SIZE = 64
NBLK = LK // BLOCK_SIZE         # 32 logical blocks per sequence
TOTAL_BLOCKS = NUM_SEQS * NBLK  # 64 physical blocks
NCH = LK // 128                 # 16 128-token kv chunks per sequence
NQT = LQ // 128                 # 8 128-token q tiles per sequence
NU = NUM_SEQS * GROUP           # 8 (seq, head) units per core
SCALE = 1.0 / float(np.sqrt(HEAD_DIM))

# exp engine split: these kv chunks go to DVE (Schraudolph bit-trick),
# the rest to ACT (exact). ~24% of exp columns on DVE.
DVE_KT = frozenset({2, 6, 10, 14})
# unit 0 (pipeline fill) alternates engines for exp latency, not accuracy
U0_DVE_KT = frozenset({1, 3, 5, 7, 9, 11, 13, 15})
# Schraudolph fp16 exp: es = bitcast_f16(int16(s * EXP_A + EXP_B))
EXP_A = float(SCALE * np.log2(np.e) * 1024.0)
EXP_B = float(15 * 1024 - 45)

_CACHE = {}


def _build_program():
    from contextlib import ExitStack

    import concourse.mybir as mybir
    import concourse.tile as tile
    from concourse import bacc

    f32 = mybir.dt.float32
    f16 = mybir.dt.float16
    i16 = mybir.dt.int16

    nc = bacc.Bacc()
    qT_d = nc.dram_tensor("qT", [128, NU * LQ], f16, kind="ExternalInput")
    kT_d = nc.dram_tensor("kT", [128, NUM_SEQS * LK], f16, kind="ExternalInput")
    vP_d = nc.dram_tensor("vP", [128, NUM_SEQS * NCH * 129], f16,
                          kind="ExternalInput")
    o_d = nc.dram_tensor("out", [128, NU * NQT * 128], f32,
                         kind="ExternalOutput")

    with tile.TileContext(nc) as tc, ExitStack() as ctx:
        persist = ctx.enter_context(tc.tile_pool(name="persist", bufs=1))
        es_pool = ctx.enter_context(tc.tile_pool(name="es", bufs=3))
        ob_pool = ctx.enter_context(tc.tile_pool(name="ob", bufs=3))
        small = ctx.enter_context(tc.tile_pool(name="small", bufs=8))
        sc_ps = ctx.enter_context(tc.tile_pool(name="sc_ps", bufs=3, space="PSUM"))
        oc_ps = ctx.enter_context(tc.tile_pool(name="oc_ps", bufs=2, space="PSUM"))

        qTs = persist.tile([128, NU * LQ], f16, tag="qTs")
        kTs = persist.tile([128, NUM_SEQS * LK], f16, tag="kTs")
        vPs = persist.tile([128, NUM_SEQS * NCH * 129], f16, tag="vPs")

        # warmup: load the Exp activation table while the input DMAs run
        warm = persist.tile([128, 1], f32, tag="warm")
        warm16 = persist.tile([128, 1], f16, tag="warm16")
        nc.vector.memset(warm[:, :], 0.0)
        nc.scalar.activation(warm16[:, :], warm[:, :],
                             mybir.ActivationFunctionType.Exp, scale=SCALE)
        # dummy matmul burst: keeps PE continuously busy through the DMA
        # fill so the p-state ramp completes before the first real QK
        warmS = persist.tile([128, 1], f16, tag="warmS")
        scratch = persist.tile([128, 64], f16, tag="scratch")
        nc.vector.memset(warmS[:, :], 0.0)
        nc.vector.memset(scratch[:, :], 0.0)
        warmP = sc_ps.tile([128, 1024], f32, tag="sc")
        for _ in range(34):
            nc.tensor.matmul(warmP[0:1, 0:64], warmS[:, :], scratch[:, :],
                             start=True, stop=True)

        # input DMAs, ordered so unit 0's dependencies land first
        nc.sync.dma_start(out=kTs[:, 0:256], in_=kT_d[:, 0:256])
        nc.sync.dma_start(out=qTs[:, 0:512], in_=qT_d[:, 0:512])
        nc.sync.dma_start(out=qTs[:, 512:LQ], in_=qT_d[:, 512:LQ])
        nc.sync.dma_start(out=kTs[:, 256:LK // 2], in_=kT_d[:, 256:LK // 2])
        nc.sync.dma_start(out=kTs[:, LK // 2:LK], in_=kT_d[:, LK // 2:LK])
        nc.sync.dma_start(out=vPs[:, 0:NCH * 129], in_=vP_d[:, 0:NCH * 129])
        for h in range(1, GROUP):
            nc.sync.dma_start(out=qTs[:, h * LQ:(h + 1) * LQ],
                              in_=qT_d[:, h * LQ:(h + 1) * LQ])
        nc.sync.dma_start(out=kTs[:, LK:2 * LK], in_=kT_d[:, LK:2 * LK])
        nc.sync.dma_start(out=vPs[:, NCH * 129:2 * NCH * 129],
                          in_=vP_d[:, NCH * 129:2 * NCH * 129])
        for h in range(GROUP):
            u = GROUP + h
            nc.sync.dma_start(out=qTs[:, u * LQ:(u + 1) * LQ],
                              in_=qT_d[:, u * LQ:(u + 1) * LQ])

        def dve_kt(u):
            # unit 0 paces the pipeline fill on exp throughput: alternate
            # ACT/DVE there; steady-state units use the accuracy-tuned set
            return U0_DVE_KT if u == 0 else DVE_KT

        def emit_qk_chunk(u, kt, es):
            s = u // GROUP
            q_lo = max(0, (kt - NCH // 2) * 128)
            width = LQ - q_lo
            ps = sc_ps.tile([128, 1024], f32, tag="sc")
            off = 0
            while off < width:
                n = min(512 - off % 512, width - off)
                nc.tensor.matmul(
                    ps[:, off:off + n],
                    kTs[:, s * LK + kt * 128:s * LK + (kt + 1) * 128],
                    qTs[:, u * LQ + q_lo + off:u * LQ + q_lo + off + n],
                    start=True, stop=True)
                off += n
            dst_lo = kt * LQ + q_lo
            if kt in dve_kt(u):
                nc.vector.tensor_scalar(
                    es[:, dst_lo:(kt + 1) * LQ].bitcast(i16),
                    ps[:, 0:width], EXP_A, EXP_B,
                    mybir.AluOpType.mult, mybir.AluOpType.add)
            else:
                nc.scalar.activation(
                    es[:, dst_lo:(kt + 1) * LQ], ps[:, 0:width],
                    mybir.ActivationFunctionType.Exp, scale=SCALE)
            if kt >= NCH // 2:
                # zero the upper triangle of the diagonal 128x128 block
                nc.gpsimd.affine_select(
                    out=es[:, dst_lo:dst_lo + 128],
                    in_=es[:, dst_lo:dst_lo + 128],
                    compare_op=mybir.AluOpType.is_ge, fill=0.0,
                    base=0, pattern=[[1, 128]], channel_multiplier=-1)

        def emit_pv_qt(u, qt, es, ob8):
            s = u // GROUP
            nch_q = NCH // 2 + 1 + qt  # kv chunks 0 .. 8+qt
            po = oc_ps.tile([128, 129], f32, tag="oc")
            for c in range(nch_q):
                nc.tensor.matmul(
                    po[:, :],
                    es[:, c * LQ + qt * 128:c * LQ + (qt + 1) * 128],
                    vPs[:, (s * NCH + c) * 129:(s * NCH + c + 1) * 129],
                    start=(c == 0), stop=(c == nch_q - 1))
            rc = small.tile([128, 1], f32, tag="rc")
            nc.vector.reciprocal(rc[:, :], po[:, 128:129])
            nc.vector.tensor_scalar_mul(
                ob8[:, qt * 128:(qt + 1) * 128], po[:, 0:128], rc[:, :])

        def emit_out_dma(u, ob8, lo=0, hi=NQT):
            nc.sync.dma_start(
                out=o_d[:, (u * NQT + lo) * 128:(u * NQT + hi) * 128],
                in_=ob8[:, lo * 128:hi * 128])

        # software pipeline: QK/exp of unit u interleaved with PV of u-1;
        # the last unit's PV is folded into its own QK loop (2-chunk lag)
        # so the tail is short. Unit 1 starts PV(u0) later (kt=6) to give
        # unit 0's exp more slack during pipeline fill.
        prev = None  # (u, es, ob8)
        for u in range(NU):
            last = u == NU - 1
            lag = 6 if u == 1 else 3
            es = es_pool.tile([128, NCH * LQ], f16, tag="es")
            ob8 = ob_pool.tile([128, NQT * 128], f32, tag="ob8")
            # first `lag` chunks fill the PSUM pipeline. For unit 0 the
            # first two chunks interleave half-width matmuls so the first
            # work depends only on the earliest DMA pieces.
            if u == 0:
                ps0 = sc_ps.tile([128, 1024], f32, tag="sc")
                ps1 = sc_ps.tile([128, 1024], f32, tag="sc")
                for half in range(2):
                    for kt, ps in ((0, ps0), (1, ps1)):
                        nc.tensor.matmul(
                            ps[:, half * 512:(half + 1) * 512],
                            kTs[:, kt * 128:(kt + 1) * 128],
                            qTs[:, half * 512:(half + 1) * 512],
                            start=True, stop=True)
                    if half == 0:
                        # busywork while qT[512:1024] lands: keeps the
                        # PE p-state ramp alive through the DMA wait
                        for _ in range(16):
                            nc.tensor.matmul(
                                warmP[0:1, 0:64], warmS[:, :],
                                scratch[:, :], start=True, stop=True)
                for kt, ps in ((0, ps0), (1, ps1)):
                    if kt in dve_kt(0):
                        nc.vector.tensor_scalar(
                            es[:, kt * LQ:(kt + 1) * LQ].bitcast(i16),
                            ps[:, :], EXP_A, EXP_B,
                            mybir.AluOpType.mult, mybir.AluOpType.add)
                    else:
                        nc.scalar.activation(
                            es[:, kt * LQ:(kt + 1) * LQ], ps[:, :],
                            mybir.ActivationFunctionType.Exp, scale=SCALE)
                for kt in range(2, lag):
                    emit_qk_chunk(u, kt, es)
            else:
                for kt in range(lag):
                    emit_qk_chunk(u, kt, es)
            for kt in range(lag, NCH):
                if prev is not None and kt - lag < NQT:
                    emit_pv_qt(prev[0], kt - lag, prev[1], prev[2])
                    if kt - lag == NQT // 2 - 1:
                        emit_out_dma(prev[0], prev[2], 0, NQT // 2)
                emit_qk_chunk(u, kt, es)
                if last and kt >= 10:
                    emit_pv_qt(u, kt - 10, es, ob8)
                    if kt - 10 == NQT // 2 - 1:
                        emit_out_dma(u, ob8, 0, NQT // 2)
            if prev is not None:
                for qt in range(NCH - lag, NQT):
                    emit_pv_qt(prev[0], qt, prev[1], prev[2])
                emit_out_dma(prev[0], prev[2], NQT // 2, NQT)
            prev = (u, es, ob8)
        for qt in range(NCH - 10, NQT):
            emit_pv_qt(prev[0], qt, prev[1], prev[2])
            if qt == 6:
                emit_out_dma(prev[0], prev[2], 4, 7)
        emit_out_dma(prev[0], prev[2], 7, NQT)

    nc.compile()
    return nc


def _get_program():
    if "p" not in _CACHE:
        _CACHE["p"] = _build_program()
    return _CACHE["p"]


def _host_prep(q, k_cache, v_cache, bt):
    """Build per-core device-layout inputs (gather + shard + cast)."""
    f16 = np.float16
    # qT: [128=d, s, h, t] per core
    qf = np.ascontiguousarray(
        q.reshape(NUM_SEQS, LQ, NUM_HEADS, HEAD_DIM).transpose(3, 0, 2, 1)
    ).astype(f16)  # [128, s, H, t]
    # gather blocks in logical order
    kg = k_cache[bt.reshape(-1)]  # [64, 64, KVH, 128]
    vg = v_cache[bt.reshape(-1)]
    # kT: [KVH][128=d, s*2048 + tok]
    kT_all = np.ascontiguousarray(
        kg.reshape(NUM_SEQS, LK, NUM_KV_HEADS, HEAD_DIM).transpose(2, 3, 0, 1)
    ).astype(f16)  # [KVH, 128, s, 2048]
    # vP: [KVH][128=tok%128, (s*16+c)*129 + d], col 128 = ones
    vr = vg.reshape(NUM_SEQS, NCH, 128, NUM_KV_HEADS, HEAD_DIM)
    vP_all = np.ones((NUM_KV_HEADS, 128, NUM_SEQS, NCH, 129), dtype=f16)
    vP_all[..., 0:128] = vr.transpose(3, 2, 0, 1, 4).astype(f16)

    in_maps = []
    for c in range(NUM_KV_HEADS):
        in_maps.append({
            "qT": np.ascontiguousarray(
                qf[:, :, c * GROUP:(c + 1) * GROUP, :]).reshape(128, NU * LQ),
            "kT": kT_all[c].reshape(128, NUM_SEQS * LK),
            "vP": np.ascontiguousarray(vP_all[c]).reshape(
                128, NUM_SEQS * NCH * 129),
        })
    return in_maps


def kernel(q, k_cache, v_cache, cu_seqlens_q, cu_seqlens_k, block_tables,
           _want_trace=False):
    from concourse import bass_utils

    q = np.asarray(q, dtype=np.float32)
    k_cache = np.asarray(k_cache, dtype=np.float32)
    v_cache = np.asarray(v_cache, dtype=np.float32)
    bt = np.asarray(block_tables, dtype=np.int32)

    assert q.shape == (NUM_SEQS * LQ, NUM_HEADS, HEAD_DIM)
    assert k_cache.shape == (TOTAL_BLOCKS, BLOCK_SIZE, NUM_KV_HEADS, HEAD_DIM)
    assert v_cache.shape == (TOTAL_BLOCKS, BLOCK_SIZE, NUM_KV_HEADS, HEAD_DIM)
    assert bt.shape == (NUM_SEQS, NBLK)
    assert bt.min() >= 0

    nc = _get_program()
    in_maps = _host_prep(q, k_cache, v_cache, bt)

    res = bass_utils.run_bass_kernel_spmd(
        nc, in_maps, core_ids=list(range(NUM_KV_HEADS)),
        trace=_want_trace,
        **({"trace_cores": list(range(NUM_KV_HEADS)), "stitch_traces": True}
           if _want_trace else {}),
    )

    out = np.empty((NUM_SEQS * LQ, NUM_HEADS, HEAD_DIM), dtype=np.float32)
    for c in range(NUM_KV_HEADS):
        # device layout [128=t, ((s*4+h)*8 + qt)*128 + d]
        r = res.results[c]["out"].reshape(128, NUM_SEQS, GROUP, NQT, HEAD_DIM)
        out[:, c * GROUP:(c + 1) * GROUP, :] = (
            r.transpose(1, 3, 0, 2, 4).reshape(NUM_SEQS * LQ, GROUP, HEAD_DIM))

    if _want_trace:
        return out, res
    return out
